# revision 1
# baseline (speedup 1.0000x reference)
"""Trainium2 Bass kernel for the quantized LM-head (nn_LmHeadTender).

Math (per core, vocab-sharded):
  reference computes
    Wl   = dequant_int4(lm_weight)            # per-row scale sw = rowmax/7
    y    = dequant_int4(x, per-(chunk,channel) scale s = tmax*2^(bucket-13)/7)
    out  = y @ Wl.T
  We factor every scale out of the matmul so that both matmul operands are
  small integers (times powers of two) that are EXACTLY representable in
  bf16:
    qw  in [-8, 7]                 (weight int values)
    yq  = qx * 2^(bucket-13)       (activation ints scaled by a power of 2)
    out[t, v] = (tmax_c/7) * sw[v] * sum_h yq[t, h] * qw[v, h]
  The bf16 matmul therefore computes exact products accumulated in fp32
  PSUM - the result matches the f32 reference to ~1e-6 (accumulation
  order), at bf16 matmul speed.

Sharding: lm_weight split into 8 vocab shards of 4000 rows, zero-padded to
4096.  hidden_states replicated.  Host concatenates the per-core [4096,
4096] logits (first 4000 cols valid) along vocab.
"""

import os
import sys
from contextlib import ExitStack

import numpy as np

import concourse.bass as bass
import concourse.tile as tile
from concourse import bacc, masks, mybir
from concourse.bass_utils import run_bass_kernel_spmd

FP = mybir.dt.float32
BF = mybir.dt.bfloat16
I32 = mybir.dt.int32
ALU = mybir.AluOpType
AX = mybir.AxisListType

T = 4096            # tokens (2*2048)
H = 4096            # hidden
V = 32000           # vocab
NCORE = 8
VSH = V // NCORE    # 4000 valid vocab rows per core
VP = 4096           # padded per-core vocab
CHUNK = 256
NCHUNK = T // CHUNK  # 16
DECOMP = 14
QMAX = 7.0
C_MAGIC = 12582912.0   # 1.5 * 2^23: round-to-nearest-even via add/sub
C7 = float(np.float32(1.0) / np.float32(7.0))  # fl(1/7); DVE has no divide op

KT = H // 128       # 32 k tiles
MT = VP // 128      # 32 weight row tiles
NT_GROUPS = 2       # token groups for the matmul phase
TG = T // (NT_GROUPS * 128)   # 16 token tiles (of 128) per group
VB = VP // 512      # 8 vocab blocks of 512


def _emit(ctx: ExitStack, tc: "tile.TileContext", x_d, w_d, out_d):
    nc = tc.nc

    # ---------------- persistent tiles ----------------
    cpool = ctx.enter_context(tc.tile_pool(name="consts", bufs=1))
    ident = cpool.tile([128, 128], FP)
    masks.make_identity(nc, ident[:])
    ones_row = cpool.tile([1, 128], FP)
    nc.vector.memset(ones_row[:], 1.0)
    sw_pk = cpool.tile([128, 32], FP)      # sw packed [p, m]; v = m*128+p
    sw_t = cpool.tile([32, 128], FP)       # sw transposed [m, p]
    sw_row = cpool.tile([1, VP], FP)       # sw on one partition, v-major
    sw_rep = cpool.tile([128, VP], FP)     # sw replicated on all partitions
    m7_all = cpool.tile([128, 16], FP)     # tmax_c/7 broadcast, col per chunk

    dpool = ctx.enter_context(tc.tile_pool(name="dram", bufs=1, space="DRAM"))
    qw_d = dpool.tile([VP, H], BF)         # quantized weight ints, [v, h]
    y_d = dpool.tile([H, T], BF)           # quantized act * 2^(b-13), [h, t]
    sw_d = dpool.tile([32, 128], FP)       # sw bounce buffer (row-major = v)

    # ---------------- weight phase ----------------
    with (
        tc.tile_pool(name="wq", bufs=2) as wq_pool,
        tc.tile_pool(name="wsm", bufs=2) as ws_pool,
    ):
        for m in range(MT):
            w_nat = wq_pool.tile([128, H], FP, tag="w_nat")
            nc.sync.dma_start(w_nat[:], w_d[m * 128:(m + 1) * 128, :])
            rmax = ws_pool.tile([128, 1], FP, tag="rmax")
            nc.vector.tensor_reduce(
                rmax[:], w_nat[:], axis=AX.X, op=ALU.max,
                apply_absolute_value=True)
            # sw = max(rmax*(1/7), 1e-9)  (reference: max(rmax/7, 1e-9))
            nc.vector.tensor_scalar(
                sw_pk[:, m:m + 1], rmax[:], C7, 1e-9, ALU.mult, ALU.max)
            rw = ws_pool.tile([128, 1], FP, tag="rw")
            nc.vector.reciprocal(rw[:], sw_pk[:, m:m + 1])
            # quantize in place: round(w*rw) clamped to [-8, 7]
            nc.vector.tensor_scalar(
                w_nat[:], w_nat[:], rw[:], C_MAGIC, ALU.mult, ALU.add)
            nc.vector.tensor_scalar(
                w_nat[:], w_nat[:], C_MAGIC, QMAX, ALU.subtract, ALU.min)
            qw_st = wq_pool.tile([128, H], BF, tag="qw_st")
            nc.vector.tensor_scalar(
                qw_st[:], w_nat[:], -(QMAX + 1.0), None, ALU.max)
            nc.sync.dma_start(qw_d[m * 128:(m + 1) * 128, :], qw_st[:])

    # ---------------- sw_rep build ----------------
    with tc.tile_pool(name="swps", bufs=4, space="PSUM") as swps_pool:
        for a in range(4):
            nc.vector.transpose(
                sw_t[:, a * 32:(a + 1) * 32], sw_pk[a * 32:(a + 1) * 32, :])
        nc.sync.dma_start(sw_d[:, :], sw_t[:])
        nc.sync.dma_start(sw_row[:], sw_d[:, :])
        for j in range(VP // 512):
            bp = swps_pool.tile([128, 512], FP, tag="bp")
            nc.tensor.matmul(
                bp[:], ones_row[:], sw_row[:, j * 512:(j + 1) * 512],
                start=True, stop=True)
            nc.scalar.copy(sw_rep[:, j * 512:(j + 1) * 512], bp[:])

    # ---------------- activation phase ----------------
    with (
        tc.tile_pool(name="xin", bufs=3) as xin_pool,
        tc.tile_pool(name="xT", bufs=2) as xT_pool,
        tc.tile_pool(name="xst", bufs=2) as st_pool,
        tc.tile_pool(name="yst", bufs=8) as y_pool,
        tc.tile_pool(name="xps", bufs=6, space="PSUM") as xps_pool,
        tc.tile_pool(name="bps", bufs=2, space="PSUM") as bps_pool,
    ):
        for c in range(NCHUNK):
            xT = xT_pool.tile([128, KT, CHUNK], FP, tag="xT")
            for th in range(2):
                xnat = xin_pool.tile([128, H], FP, tag="xn")
                nc.sync.dma_start(
                    xnat[:],
                    x_d[c * CHUNK + th * 128: c * CHUNK + (th + 1) * 128, :])
                for i in range(KT):
                    pst = xps_pool.tile([128, 128], FP, tag="pst")
                    nc.tensor.transpose(
                        pst[:], xnat[:, i * 128:(i + 1) * 128], ident[:])
                    dst = xT[:, i, th * 128:(th + 1) * 128]
                    if (i + th) % 2 == 0:
                        nc.scalar.copy(dst, pst[:])
                    else:
                        nc.vector.tensor_copy(dst, pst[:])
            # ---- stats: cmax per channel, tmax per chunk ----
            cmax = st_pool.tile([128, KT], FP, tag="cmax")
            nc.vector.tensor_reduce(
                cmax[:], xT[:], axis=AX.X, op=ALU.max,
                apply_absolute_value=True)
            tpad = st_pool.tile([128, 32], FP, tag="tpad")
            nc.vector.memset(tpad[:], 0.0)
            nc.vector.tensor_reduce(
                tpad[:, 0:1], cmax[:], axis=AX.X, op=ALU.max)
            tt = st_pool.tile([32, 128], FP, tag="tt")
            for a in range(4):
                nc.vector.transpose(
                    tt[:, a * 32:(a + 1) * 32], tpad[a * 32:(a + 1) * 32, :])
            tmax_sc = st_pool.tile([1, 1], FP, tag="tmax_sc")
            nc.vector.tensor_reduce(
                tmax_sc[:], tt[0:1, :], axis=AX.X, op=ALU.max)
            # broadcast tmax to 128 partitions via PE outer product
            bp1 = bps_pool.tile([128, 1], FP, tag="bp1")
            nc.tensor.matmul(
                bp1[:], ones_row[:], tmax_sc[:], start=True, stop=True)
            tmax_b = st_pool.tile([128, 1], FP, tag="tmax_b")
            nc.scalar.copy(tmax_b[:], bp1[:])
            nc.vector.tensor_scalar(
                m7_all[:, c:c + 1], tmax_b[:], C7, None, ALU.mult)
            # ---- bucket: number of thresholds strictly exceeded ----
            bucket = st_pool.tile([128, KT], FP, tag="bucket")
            nc.vector.memset(bucket[:], 0.0)
            for lv in range(DECOMP - 1):
                thr = st_pool.tile([128, 1], FP, tag="thr", bufs=2)
                nc.vector.tensor_scalar(
                    thr[:], tmax_b[:], 2.0 ** (lv - (DECOMP - 1)), None,
                    ALU.mult)
                nc.vector.scalar_tensor_tensor(
                    bucket[:], cmax[:], thr[:], bucket[:],
                    op0=ALU.is_gt, op1=ALU.add)
            # ---- pw = 2^(bucket-13) exactly, via IEEE bit construction ----
            g = st_pool.tile([128, KT], FP, tag="g")
            nc.vector.tensor_scalar(
                g[:], bucket[:], 114.0, 8388608.0, ALU.add, ALU.mult)
            g_i = st_pool.tile([128, KT], I32, tag="g_i")
            nc.vector.tensor_copy(g_i[:], g[:])
            pw = g_i[:].bitcast(FP)
            # ---- scales: s = max(tmax*pw/7, 1e-9); r = 1/s ----
            ch_thr = st_pool.tile([128, KT], FP, tag="ch_thr")
            nc.vector.tensor_scalar(
                ch_thr[:], pw, tmax_b[:], None, ALU.mult)
            s_t = st_pool.tile([128, KT], FP, tag="s_t")
            nc.vector.tensor_scalar(
                s_t[:], ch_thr[:], C7, 1e-9, ALU.mult, ALU.max)
            r_t = st_pool.tile([128, KT], FP, tag="r_t")
            nc.vector.reciprocal(r_t[:], s_t[:])
            # ---- quantize: y = clip(round(x*r), -8, 7) * pw  (bf16) ----
            for i in range(KT):
                sl = xT[:, i, :]
                nc.vector.tensor_scalar(
                    sl, sl, r_t[:, i:i + 1], C_MAGIC, ALU.mult, ALU.add)
                nc.vector.tensor_scalar(
                    sl, sl, C_MAGIC, QMAX, ALU.subtract, ALU.min)
                y_st = y_pool.tile([128, CHUNK], BF, tag="y_st")
                pw_col = g_i[:, i:i + 1].bitcast(FP)
                nc.vector.tensor_scalar(
                    y_st[:], sl, -(QMAX + 1.0), pw_col, ALU.max, ALU.mult)
                nc.sync.dma_start(
                    y_d[i * 128:(i + 1) * 128, c * CHUNK:(c + 1) * CHUNK],
                    y_st[:])

    # ---------------- matmul phase ----------------
    with (
        tc.tile_pool(name="ymm", bufs=1) as ymm_pool,
        tc.tile_pool(name="qwp", bufs=KT + 2) as qw_pool,
        tc.tile_pool(name="stg", bufs=4) as stg_pool,
        tc.tile_pool(name="mps", bufs=8, space="PSUM") as mps_pool,
    ):
        tok_g = TG * 128  # tokens per group
        for grp in range(NT_GROUPS):
            y_all = ymm_pool.tile([128, KT, tok_g], BF, tag="y_all")
            for k in range(KT):
                nc.sync.dma_start(
                    y_all[:, k, :],
                    y_d[k * 128:(k + 1) * 128,
                        grp * tok_g:(grp + 1) * tok_g])
            for vb in range(VB):
                qwt = []
                for k in range(KT):
                    qt = qw_pool.tile([128, 512], BF, tag="qw")
                    nc.sync.dma_start(
                        qt[:],
                        qw_d[vb * 512:(vb + 1) * 512,
                             k * 128:(k + 1) * 128],
                        transpose=True)
                    qwt.append(qt)
                for t in range(TG):
                    ps = mps_pool.tile([128, 512], FP, tag="ps")
                    for k in range(KT):
                        nc.tensor.matmul(
                            ps[:],
                            y_all[:, k, t * 128:(t + 1) * 128],
                            qwt[k][:],
                            start=(k == 0), stop=(k == KT - 1))
                    stg = stg_pool.tile([128, 512], FP, tag="stg")
                    tidx = grp * TG + t
                    cch = (tidx * 128) // CHUNK
                    nc.vector.scalar_tensor_tensor(
                        stg[:], ps[:], m7_all[:, cch:cch + 1],
                        sw_rep[:, vb * 512:(vb + 1) * 512],
                        op0=ALU.mult, op1=ALU.mult)
                    nc.sync.dma_start(
                        out_d[tidx * 128:(tidx + 1) * 128,
                              vb * 512:(vb + 1) * 512],
                        stg[:])


_CACHED = None


def _build():
    global _CACHED
    if _CACHED is not None:
        return _CACHED
    nc = bacc.Bacc(
        "TRN2", target_bir_lowering=False, debug=False,
        enable_asserts=False, num_devices=NCORE)
    x_d = nc.dram_tensor("x", (T, H), FP, kind="ExternalInput").ap()
    w_d = nc.dram_tensor("w", (VP, H), FP, kind="ExternalInput").ap()
    out_d = nc.dram_tensor("out", (T, VP), FP, kind="ExternalOutput").ap()
    with tile.TileContext(nc) as tc:
        with ExitStack() as ctx:
            _emit(ctx, tc, x_d, w_d, out_d)
    nc.compile()
    _CACHED = nc
    return nc


def kernel(hidden_states: np.ndarray, lm_weight: np.ndarray) -> np.ndarray:
    b, t, h = hidden_states.shape
    assert (b * t, h) == (T, H) and lm_weight.shape == (V, H)
    x_full = np.ascontiguousarray(
        hidden_states.reshape(T, H).astype(np.float32))
    in_maps = []
    for c in range(NCORE):
        shard = np.zeros((VP, H), dtype=np.float32)
        shard[:VSH] = lm_weight[c * VSH:(c + 1) * VSH]
        in_maps.append({"x": x_full, "w": shard})
    nc = _build()
    res = run_bass_kernel_spmd(nc, in_maps, core_ids=list(range(NCORE)))
    outs = [res.results[c]["out"][:, :VSH] for c in range(NCORE)]
    full = np.concatenate(outs, axis=1)
    return full.reshape(b, t, V)



# revision 3
# speedup vs baseline: 2.0823x; 2.0823x over previous
"""Trainium2 Bass kernel for the quantized LM-head (nn_LmHeadTender).

fp8 (e5m2) DoubleRow implementation.

Math (per core, vocab-sharded; vocab shard = 4000 rows, no padding):
    Wl   = dequant_int4(lm_weight)          # per-row scale sw = rowmax/7
    y    = dequant_int4(x, per-(chunk,channel) scale s = tmax*2^(b-13)/7)
    out  = y @ Wl.T
Every scale is factored out of the matmul so both operands are exactly
representable in fp8 e5m2:
    qw  in [-7, 7]             (weight ints; |w/s| <= 7 by construction,
                                so no clamping is ever needed)
    yq  = qx * 2^(bucket-13)   (activation ints scaled by a power of 2;
                                min magnitude 2^-13 >= e5m2 min normal 2^-14)
    out[t, v] = (tmax_c/7) * sw[v] * sum_h yq[t, h] * qw[v, h]
The e5m2 DoubleRow matmul (2 fp8 MACs/cell/cycle) computes exact products
accumulated in fp32 PSUM.

Structure: weight phase quantizes lm_weight and leaves qw^T resident in
SBUF as fp8 ([128, 32, 4000] = 125 KiB/partition).  The main loop is
software-pipelined per 256-token chunk: DMA x -> PE transpose -> stats +
quantize (spread across scalar/vector/pool engines) -> DoubleRow matmuls
-> scaled drain -> DMA out.  No DRAM bounce for y or qw.
"""

import numpy as np
from contextlib import ExitStack

import concourse.bass as bass
import concourse.tile as tile
from concourse import bacc, masks, mybir
from concourse.bass_utils import run_bass_kernel_spmd

FP = mybir.dt.float32
BF = mybir.dt.bfloat16
F8 = mybir.dt.float8e5
I32 = mybir.dt.int32
ALU = mybir.AluOpType
AX = mybir.AxisListType
ACT = mybir.ActivationFunctionType
DR = mybir.MatmulPerfMode.DoubleRow

T = 4096            # tokens (2*2048)
H = 4096            # hidden
V = 32000           # vocab
NCORE = 8
VSH = V // NCORE    # 4000 vocab rows per core (exact, no padding)
CHUNK = 256
NCHUNK = T // CHUNK  # 16
KT = H // 128       # 32 k tiles
KP = KT // 2        # 16 k pairs (DoubleRow)
VBS = 500           # vocab block size (one PSUM bank holds 512 fp32)
VB = VSH // VBS     # 8 blocks
MT = 32             # weight row tiles: 31 x 128 + 1 x 32
DECOMP = 14
QMAX = 7.0
C_MAGIC = 12582912.0   # 1.5 * 2^23: round-to-nearest-even via add/sub
C7 = float(np.float32(1.0) / np.float32(7.0))  # fl(1/7); no DVE divide


def _emit(ctx: ExitStack, tc: "tile.TileContext", x_d, w_d, out_d):
    nc = tc.nc

    # ---------------- persistent tiles ----------------
    cpool = ctx.enter_context(tc.tile_pool(name="consts", bufs=1))
    ident = cpool.tile([128, 128], FP)
    masks.make_identity(nc, ident[:])
    ident_bf = cpool.tile([128, 128], BF)
    masks.make_identity(nc, ident_bf[:])
    ones_row = cpool.tile([1, 128], FP)
    nc.vector.memset(ones_row[:], 1.0)
    sw_pk = cpool.tile([128, 32], FP)      # sw packed [p, m]; v = m*128+p
    nc.vector.memset(sw_pk[:], 0.0)        # last tile fills only 32 rows
    sw_t = cpool.tile([32, 128], FP)       # sw transposed [m, p]
    sw_rep = cpool.tile([128, VSH], FP)    # sw replicated on all partitions
    m7_all = cpool.tile([128, NCHUNK], FP)  # tmax_c/7, col per chunk
    qw_sb = cpool.tile([128, KT, VSH], F8)  # quantized weight^T, resident

    dpool = ctx.enter_context(tc.tile_pool(name="dram", bufs=1, space="DRAM"))
    sw_d = dpool.tile([32, 128], FP)       # sw bounce buffer (row-major = v)

    # ---------------- weight phase ----------------
    # per 128-row tile: rowmax (vector) -> sw, rw; round via magic const
    # (scalar ACTIVATE w*rw + MAGIC, then vector subtract -> bf16 ints);
    # PE-transpose to [h, v]; pool-engine copy psum -> fp8 qw_sb.
    with (
        tc.tile_pool(name="wq", bufs=2) as wq_pool,
        tc.tile_pool(name="wqi", bufs=2) as wqi_pool,
        tc.tile_pool(name="wsm", bufs=2) as ws_pool,
        tc.tile_pool(name="wps", bufs=4, space="PSUM") as wps_pool,
    ):
        for m in range(MT):
            rows = 128 if m < MT - 1 else VSH - 128 * (MT - 1)  # 32 for last
            w_nat = wq_pool.tile([128, H], FP, tag="w_nat")
            nc.sync.dma_start(w_nat[:rows], w_d[m * 128:m * 128 + rows, :])
            rmax = ws_pool.tile([128, 1], FP, tag="rmax")
            nc.vector.tensor_reduce(
                rmax[:rows], w_nat[:rows], axis=AX.X, op=ALU.max,
                apply_absolute_value=True)
            # sw = max(rmax*(1/7), 1e-9)  (reference: max(rmax/7, 1e-9))
            nc.vector.tensor_scalar(
                sw_pk[:rows, m:m + 1], rmax[:rows], C7, 1e-9,
                ALU.mult, ALU.max)
            rw = ws_pool.tile([128, 1], FP, tag="rw")
            nc.vector.reciprocal(rw[:rows], sw_pk[:rows, m:m + 1])
            # round(w*rw): |w*rw| <= 7 so no clamp needed
            nc.scalar.activation(
                w_nat[:rows], w_nat[:rows], ACT.Copy,
                bias=C_MAGIC, scale=rw[:rows])
            qi = wqi_pool.tile([128, H], BF, tag="qi")
            nc.vector.tensor_scalar(
                qi[:rows], w_nat[:rows], C_MAGIC, None, ALU.subtract)
            for g in range(KT // 4):
                ps = wps_pool.tile([128, 512], BF, tag="wps")
                for q in range(4):
                    kt = g * 4 + q
                    nc.tensor.transpose(
                        ps[:, q * rows:(q + 1) * rows],
                        qi[:rows, kt * 128:(kt + 1) * 128],
                        ident_bf[:rows, :rows])
                dst = qw_sb[:, g * 4:g * 4 + 4, m * 128:m * 128 + rows]
                src = ps[:, 0:4 * rows].rearrange("p (a b) -> p a b", a=4)
                if g % 2 == 0:
                    nc.scalar.copy(dst, src)
                else:
                    nc.vector.tensor_copy(dst, src)

    # ---------------- sw_rep build ----------------
    with (
        tc.tile_pool(name="swrow", bufs=1) as swrow_pool,
        tc.tile_pool(name="swps", bufs=2, space="PSUM") as swps_pool,
    ):
        sw_row = swrow_pool.tile([1, VSH], FP)  # sw on partition 0, v-major
        for a in range(4):
            nc.vector.transpose(
                sw_t[:, a * 32:(a + 1) * 32], sw_pk[a * 32:(a + 1) * 32, :])
        nc.sync.dma_start(sw_d[:, :], sw_t[:])
        # first 4000 of the 4096 row-major entries are the valid sw values
        nc.sync.dma_start(sw_row[:, 0:3968], sw_d[0:31, :])
        nc.sync.dma_start(sw_row[:, 3968:VSH], sw_d[31:32, 0:32])
        for j in range(VB):
            bp = swps_pool.tile([128, 512], FP, tag="bp")
            nc.tensor.matmul(
                bp[:, 0:VBS], ones_row[:], sw_row[:, j * VBS:(j + 1) * VBS],
                start=True, stop=True)
            nc.scalar.copy(sw_rep[:, j * VBS:(j + 1) * VBS], bp[:, 0:VBS])

    # ---------------- fused activation + matmul loop ----------------
    with (
        tc.tile_pool(name="xin", bufs=3) as xin_pool,
        tc.tile_pool(name="xT", bufs=1) as xT_pool,
        tc.tile_pool(name="yq", bufs=2) as y_pool,
        tc.tile_pool(name="xst", bufs=2) as st_pool,
        tc.tile_pool(name="stg", bufs=2) as stg_pool,
        tc.tile_pool(name="xps", bufs=3, space="PSUM") as xps_pool,
        tc.tile_pool(name="mps", bufs=4, space="PSUM") as mps_pool,
    ):
        y_tiles = {}

        def emit_x(c):
            xT = xT_pool.tile([128, KT, CHUNK], FP, tag="xT")
            y_c = y_pool.tile([128, KT, CHUNK], F8, tag="y")
            y_tiles[c] = y_c
            # transpose x into [h, t] layout via PE, 4 k-tiles per PSUM tile
            for th in range(2):
                base = c * CHUNK + th * 128
                for g in range(KT // 4):
                    xa = xin_pool.tile([128, 256], FP, tag="xn")
                    xb = xin_pool.tile([128, 256], FP, tag="xn")
                    nc.sync.dma_start(
                        xa[:], x_d[base:base + 128,
                                   (4 * g) * 128:(4 * g + 2) * 128])
                    nc.sync.dma_start(
                        xb[:], x_d[base:base + 128,
                                   (4 * g + 2) * 128:(4 * g + 4) * 128])
                    ps = xps_pool.tile([128, 512], FP, tag="xps")
                    nc.tensor.transpose(ps[:, 0:128], xa[:, 0:128], ident[:])
                    nc.tensor.transpose(ps[:, 128:256], xa[:, 128:256],
                                        ident[:])
                    nc.tensor.transpose(ps[:, 256:384], xb[:, 0:128],
                                        ident[:])
                    nc.tensor.transpose(ps[:, 384:512], xb[:, 128:256],
                                        ident[:])
                    dst = xT[:, 4 * g:4 * g + 4, th * 128:(th + 1) * 128]
                    src = ps[:].rearrange("p (a b) -> p a b", a=4)
                    if g % 2 == 0:
                        nc.scalar.copy(dst, src)
                    else:
                        nc.vector.tensor_copy(dst, src)
            # ---- stats: cmax per channel, tmax per chunk ----
            cmax = st_pool.tile([128, KT], FP, tag="cmax")
            nc.vector.tensor_reduce(
                cmax[:], xT[:], axis=AX.X, op=ALU.max,
                apply_absolute_value=True)
            tpad = st_pool.tile([128, 32], FP, tag="tpad")
            nc.vector.memset(tpad[:], 0.0)
            nc.vector.tensor_reduce(
                tpad[:, 0:1], cmax[:], axis=AX.X, op=ALU.max)
            tt = st_pool.tile([32, 128], FP, tag="tt")
            for a in range(4):
                nc.vector.transpose(
                    tt[:, a * 32:(a + 1) * 32], tpad[a * 32:(a + 1) * 32, :])
            tmax_sc = st_pool.tile([1, 1], FP, tag="tmax_sc")
            nc.vector.tensor_reduce(
                tmax_sc[:], tt[0:1, :], axis=AX.X, op=ALU.max)
            tmax_b = st_pool.tile([128, 1], FP, tag="tmax_b")
            nc.gpsimd.partition_broadcast(tmax_b[:], tmax_sc[:])
            nc.vector.tensor_scalar(
                m7_all[:, c:c + 1], tmax_b[:], C7, None, ALU.mult)
            # ---- bucket: number of thresholds strictly exceeded ----
            bucket = st_pool.tile([128, KT], FP, tag="bucket")
            nc.vector.memset(bucket[:], 0.0)
            for lv in range(DECOMP - 1):
                thr = st_pool.tile([128, 1], FP, tag="thr", bufs=2)
                nc.vector.tensor_scalar(
                    thr[:], tmax_b[:], 2.0 ** (lv - (DECOMP - 1)), None,
                    ALU.mult)
                nc.vector.scalar_tensor_tensor(
                    bucket[:], cmax[:], thr[:], bucket[:],
                    op0=ALU.is_gt, op1=ALU.add)
            # ---- pw = 2^(bucket-13) exactly, via IEEE bit construction ----
            g_f = st_pool.tile([128, KT], FP, tag="g_f")
            nc.vector.tensor_scalar(
                g_f[:], bucket[:], 114.0, 8388608.0, ALU.add, ALU.mult)
            g_i = st_pool.tile([128, KT], I32, tag="g_i")
            nc.vector.tensor_copy(g_i[:], g_f[:])
            pw = g_i[:].bitcast(FP)
            # ---- scales: s = max(tmax*pw/7, 1e-9); r = 1/s ----
            ch_thr = st_pool.tile([128, KT], FP, tag="ch_thr")
            nc.vector.tensor_scalar(
                ch_thr[:], pw, tmax_b[:], None, ALU.mult)
            s_t = st_pool.tile([128, KT], FP, tag="s_t")
            nc.vector.tensor_scalar(
                s_t[:], ch_thr[:], C7, 1e-9, ALU.mult, ALU.max)
            r_t = st_pool.tile([128, KT], FP, tag="r_t")
            nc.vector.reciprocal(r_t[:], s_t[:])
            # ---- quantize: y = round(x*r) * pw   (fp8 e5m2, exact) ----
            # |x*r| <= 7 by construction so no clamping is needed
            for kt in range(KT):
                nc.scalar.activation(
                    xT[:, kt, :], xT[:, kt, :], ACT.Copy,
                    bias=C_MAGIC, scale=r_t[:, kt:kt + 1])
            for kt in range(KT):
                nc.vector.tensor_scalar(
                    y_c[:, kt, :], xT[:, kt, :], C_MAGIC,
                    g_i[:, kt:kt + 1].bitcast(FP), ALU.subtract, ALU.mult)

        def emit_m(c):
            y_c = y_tiles.pop(c)
            for th in range(2):
                tt_idx = c * 2 + th
                for vh in range(2):
                    pss = []
                    for q in range(4):
                        ps_mm = mps_pool.tile([128, 512], FP, tag="mps")
                        pss.append(ps_mm)
                    for kp in range(KP):
                        lhsT = y_c[:, 2 * kp:2 * kp + 2,
                                   th * 128:(th + 1) * 128]
                        for q in range(4):
                            vb = vh * 4 + q
                            nc.tensor.matmul(
                                pss[q][:, 0:VBS], lhsT,
                                qw_sb[:, 2 * kp:2 * kp + 2,
                                      vb * VBS:(vb + 1) * VBS],
                                start=(kp == 0), stop=(kp == KP - 1),
                                perf_mode=DR)
                    for q in range(4):
                        vb = vh * 4 + q
                        stg = stg_pool.tile([128, VBS], FP, tag="stg")
                        nc.vector.scalar_tensor_tensor(
                            stg[:], pss[q][:, 0:VBS], m7_all[:, c:c + 1],
                            sw_rep[:, vb * VBS:(vb + 1) * VBS],
                            op0=ALU.mult, op1=ALU.mult)
                        nc.sync.dma_start(
                            out_d[tt_idx * 128:(tt_idx + 1) * 128,
                                  vb * VBS:(vb + 1) * VBS],
                            stg[:])

        emit_x(0)
        emit_x(1)
        for c in range(NCHUNK):
            emit_m(c)
            if c + 2 < NCHUNK:
                emit_x(c + 2)


_CACHED = None


def _build():
    global _CACHED
    if _CACHED is not None:
        return _CACHED
    nc = bacc.Bacc(
        "TRN2", target_bir_lowering=False, debug=False,
        enable_asserts=False, num_devices=NCORE)
    x_d = nc.dram_tensor("x", (T, H), FP, kind="ExternalInput").ap()
    w_d = nc.dram_tensor("w", (VSH, H), FP, kind="ExternalInput").ap()
    out_d = nc.dram_tensor("out", (T, VSH), FP, kind="ExternalOutput").ap()
    with tile.TileContext(nc) as tc:
        with ExitStack() as ctx:
            _emit(ctx, tc, x_d, w_d, out_d)
    nc.compile()
    _CACHED = nc
    return nc


def kernel(hidden_states: np.ndarray, lm_weight: np.ndarray) -> np.ndarray:
    b, t, h = hidden_states.shape
    assert (b * t, h) == (T, H) and lm_weight.shape == (V, H)
    x_full = np.ascontiguousarray(
        hidden_states.reshape(T, H).astype(np.float32))
    in_maps = []
    for c in range(NCORE):
        shard = np.ascontiguousarray(
            lm_weight[c * VSH:(c + 1) * VSH].astype(np.float32))
        in_maps.append({"x": x_full, "w": shard})
    nc = _build()
    res = run_bass_kernel_spmd(nc, in_maps, core_ids=list(range(NCORE)))
    outs = [res.results[c]["out"] for c in range(NCORE)]
    full = np.concatenate(outs, axis=1)
    return full.reshape(b, t, V)


# revision 5
# speedup vs baseline: 2.2585x; 1.0846x over previous
"""Trainium2 Bass kernel for the quantized LM-head (nn_LmHeadTender).

fp8 (e5m2) DoubleRow implementation.

Math (per core, vocab-sharded; vocab shard = 4000 rows, no padding):
    Wl   = dequant_int4(lm_weight)          # per-row scale sw = rowmax/7
    y    = dequant_int4(x, per-(chunk,channel) scale s = tmax*2^(b-13)/7)
    out  = y @ Wl.T
Every scale is factored out of the matmul so both operands are exactly
representable in fp8 e5m2:
    qw  in [-7, 7]             (weight ints; |w/s| <= 7 by construction,
                                so no clamping is ever needed)
    yq  = qx * 2^(bucket-13)   (activation ints scaled by a power of 2;
                                min magnitude 2^-13 >= e5m2 min normal 2^-14)
    out[t, v] = (tmax_c/7) * sw[v] * sum_h yq[t, h] * qw[v, h]
The e5m2 DoubleRow matmul (2 fp8 MACs/cell/cycle) computes exact products
accumulated in fp32 PSUM.

Structure: weight phase quantizes lm_weight and leaves qw^T resident in
SBUF as fp8 ([128, 32, 4000] = 125 KiB/partition).  The main loop is
software-pipelined per 256-token chunk: DMA x -> PE transpose -> stats +
quantize (spread across scalar/vector/pool engines) -> DoubleRow matmuls
-> scaled drain -> DMA out.  No DRAM bounce for y or qw.
"""

import numpy as np
from contextlib import ExitStack

import concourse.bass as bass
import concourse.tile as tile
from concourse import bacc, bass_isa, masks, mybir
from concourse.bass_utils import run_bass_kernel_spmd

FP = mybir.dt.float32
BF = mybir.dt.bfloat16
F8 = mybir.dt.float8e5
I32 = mybir.dt.int32
ALU = mybir.AluOpType
AX = mybir.AxisListType
ACT = mybir.ActivationFunctionType
DR = mybir.MatmulPerfMode.DoubleRow

T = 4096            # tokens (2*2048)
H = 4096            # hidden
V = 32000           # vocab
NCORE = 8
VSH = V // NCORE    # 4000 vocab rows per core (exact, no padding)
CHUNK = 256
NCHUNK = T // CHUNK  # 16
KT = H // 128       # 32 k tiles
KP = KT // 2        # 16 k pairs (DoubleRow)
VBS = 500           # vocab block size (one PSUM bank holds 512 fp32)
VB = VSH // VBS     # 8 blocks
MT = 32             # weight row tiles: 31 x 128 + 1 x 32
DECOMP = 14
QMAX = 7.0
C_MAGIC = 12582912.0   # 1.5 * 2^23: round-to-nearest-even via add/sub
C7 = float(np.float32(1.0) / np.float32(7.0))  # fl(1/7); no DVE divide


def _emit(ctx: ExitStack, tc: "tile.TileContext", x_d, w_d, out_d):
    nc = tc.nc

    # ---------------- persistent tiles ----------------
    cpool = ctx.enter_context(tc.tile_pool(name="consts", bufs=1))
    ident = cpool.tile([128, 128], FP)
    masks.make_identity(nc, ident[:])
    ident_bf = cpool.tile([128, 128], BF)
    masks.make_identity(nc, ident_bf[:])
    ones_row = cpool.tile([1, 128], FP)
    nc.vector.memset(ones_row[:], 1.0)
    sw_pk = cpool.tile([128, 32], FP)      # sw packed [p, m]; v = m*128+p
    nc.vector.memset(sw_pk[:], 0.0)        # last tile fills only 32 rows
    sw_t = cpool.tile([32, 128], FP)       # sw transposed [m, p]
    sw_rep = cpool.tile([128, VSH], FP)    # sw replicated on all partitions
    m7_all = cpool.tile([128, NCHUNK], FP)  # tmax_c/7, col per chunk
    qw_sb = cpool.tile([128, KT, VSH], F8)  # quantized weight^T, resident

    dpool = ctx.enter_context(tc.tile_pool(name="dram", bufs=1, space="DRAM"))
    sw_d = dpool.tile([32, 128], FP)       # sw bounce buffer (row-major = v)

    # ---------------- weight phase ----------------
    # per 128-row tile: rowmax (vector) -> sw, rw; round via magic const
    # (scalar ACTIVATE w*rw + MAGIC, then vector subtract -> bf16 ints);
    # PE-transpose to [h, v]; pool-engine copy psum -> fp8 qw_sb.
    with (
        tc.tile_pool(name="wq", bufs=2) as wq_pool,
        tc.tile_pool(name="wqi", bufs=2) as wqi_pool,
        tc.tile_pool(name="wsm", bufs=2) as ws_pool,
        tc.tile_pool(name="wps", bufs=3, space="PSUM") as wps_pool,
    ):
        for m in range(MT):
            rows = 128 if m < MT - 1 else VSH - 128 * (MT - 1)  # 32 for last
            w_nat = wq_pool.tile([128, H], FP, tag="w_nat")
            nc.sync.dma_start(w_nat[:rows], w_d[m * 128:m * 128 + rows, :])
            rmax = ws_pool.tile([128, 1], FP, tag="rmax")
            nc.vector.tensor_reduce(
                rmax[:rows], w_nat[:rows], axis=AX.X, op=ALU.max,
                apply_absolute_value=True)
            # sw = max(rmax*(1/7), 1e-9)  (reference: max(rmax/7, 1e-9))
            nc.vector.tensor_scalar(
                sw_pk[:rows, m:m + 1], rmax[:rows], C7, 1e-9,
                ALU.mult, ALU.max)
            rw = ws_pool.tile([128, 1], FP, tag="rw")
            nc.vector.reciprocal(rw[:rows], sw_pk[:rows, m:m + 1])
            # round(w*rw): |w*rw| <= 7 so no clamp needed
            nc.scalar.activation(
                w_nat[:rows], w_nat[:rows], ACT.Copy,
                bias=C_MAGIC, scale=rw[:rows])
            qi = wqi_pool.tile([128, H], BF, tag="qi")
            nc.scalar.activation(
                qi[:rows], w_nat[:rows], ACT.Copy, bias=-C_MAGIC)
            for g in range(KT // 8):
                ps = wps_pool.tile([128, 1024], BF, tag="wps")
                for q in range(8):
                    kt = g * 8 + q
                    nc.tensor.transpose(
                        ps[:, q * rows:(q + 1) * rows],
                        qi[:rows, kt * 128:(kt + 1) * 128],
                        ident_bf[:rows, :rows])
                dst = qw_sb[:, g * 8:g * 8 + 8, m * 128:m * 128 + rows]
                src = ps[:, 0:8 * rows].rearrange("p (a b) -> p a b", a=8)
                if g % 2 == 0:
                    nc.scalar.copy(dst, src)
                else:
                    nc.vector.tensor_copy(dst, src)

    # ---------------- sw_rep build ----------------
    with (
        tc.tile_pool(name="swrow", bufs=1) as swrow_pool,
        tc.tile_pool(name="swps", bufs=2, space="PSUM") as swps_pool,
    ):
        sw_row = swrow_pool.tile([1, VSH], FP)  # sw on partition 0, v-major
        for a in range(4):
            nc.vector.transpose(
                sw_t[:, a * 32:(a + 1) * 32], sw_pk[a * 32:(a + 1) * 32, :])
        nc.sync.dma_start(sw_d[:, :], sw_t[:])
        # first 4000 of the 4096 row-major entries are the valid sw values
        nc.sync.dma_start(sw_row[:, 0:3968], sw_d[0:31, :])
        nc.sync.dma_start(sw_row[:, 3968:VSH], sw_d[31:32, 0:32])
        for j in range(VB):
            bp = swps_pool.tile([128, 512], FP, tag="bp")
            nc.tensor.matmul(
                bp[:, 0:VBS], ones_row[:], sw_row[:, j * VBS:(j + 1) * VBS],
                start=True, stop=True)
            nc.scalar.copy(sw_rep[:, j * VBS:(j + 1) * VBS], bp[:, 0:VBS])

    # ---------------- fused activation + matmul loop ----------------
    with (
        tc.tile_pool(name="xin", bufs=3) as xin_pool,
        tc.tile_pool(name="xT", bufs=1) as xT_pool,
        tc.tile_pool(name="yq", bufs=2) as y_pool,
        tc.tile_pool(name="xst", bufs=2) as st_pool,
        tc.tile_pool(name="stg", bufs=2) as stg_pool,
        tc.tile_pool(name="xps", bufs=3, space="PSUM") as xps_pool,
        tc.tile_pool(name="mps", bufs=5, space="PSUM") as mps_pool,
    ):
        y_tiles = {}

        def emit_x(c):
            xT = xT_pool.tile([128, KT, CHUNK], FP, tag="xT")
            y_c = y_pool.tile([128, KT, CHUNK], F8, tag="y")
            y_tiles[c] = y_c
            # transpose x into [h, t] layout via PE, 4 k-tiles per PSUM tile
            for th in range(2):
                base = c * CHUNK + th * 128
                for g in range(KT // 4):
                    xa = xin_pool.tile([128, 512], FP, tag="xn")
                    dma_eng = nc.sync if g % 2 == 0 else nc.scalar
                    dma_eng.dma_start(
                        xa[:], x_d[base:base + 128,
                                   (4 * g) * 128:(4 * g + 4) * 128])
                    ps = xps_pool.tile([128, 512], FP, tag="xps")
                    for q in range(4):
                        nc.tensor.transpose(
                            ps[:, q * 128:(q + 1) * 128],
                            xa[:, q * 128:(q + 1) * 128], ident[:])
                    dst = xT[:, 4 * g:4 * g + 4, th * 128:(th + 1) * 128]
                    src = ps[:].rearrange("p (a b) -> p a b", a=4)
                    if g % 2 == 0:
                        nc.scalar.copy(dst, src)
                    else:
                        nc.vector.tensor_copy(dst, src)
            # ---- stats: cmax per channel, tmax per chunk ----
            cmax = st_pool.tile([128, KT], FP, tag="cmax")
            nc.vector.tensor_reduce(
                cmax[:], xT[:], axis=AX.X, op=ALU.max,
                apply_absolute_value=True)
            tpad = st_pool.tile([128, 32], FP, tag="tpad")
            nc.vector.memset(tpad[:], 0.0)
            nc.vector.tensor_reduce(
                tpad[:, 0:1], cmax[:], axis=AX.X, op=ALU.max)
            tt = st_pool.tile([32, 128], FP, tag="tt")
            for a in range(4):
                nc.vector.transpose(
                    tt[:, a * 32:(a + 1) * 32], tpad[a * 32:(a + 1) * 32, :])
            tmax_sc = st_pool.tile([1, 1], FP, tag="tmax_sc")
            nc.vector.tensor_reduce(
                tmax_sc[:], tt[0:1, :], axis=AX.X, op=ALU.max)
            tmax_b = st_pool.tile([128, 1], FP, tag="tmax_b")
            nc.gpsimd.partition_broadcast(tmax_b[:], tmax_sc[:])
            nc.vector.tensor_scalar(
                m7_all[:, c:c + 1], tmax_b[:], C7, None, ALU.mult)
            # ---- bucket: number of thresholds strictly exceeded ----
            bucket = st_pool.tile([128, KT], FP, tag="bucket")
            nc.vector.memset(bucket[:], 0.0)
            for lv in range(DECOMP - 1):
                thr = st_pool.tile([128, 1], FP, tag="thr", bufs=2)
                nc.vector.tensor_scalar(
                    thr[:], tmax_b[:], 2.0 ** (lv - (DECOMP - 1)), None,
                    ALU.mult)
                nc.vector.scalar_tensor_tensor(
                    bucket[:], cmax[:], thr[:], bucket[:],
                    op0=ALU.is_gt, op1=ALU.add)
            # ---- pw = 2^(bucket-13) exactly, via IEEE bit construction ----
            g_f = st_pool.tile([128, KT], FP, tag="g_f")
            nc.vector.tensor_scalar(
                g_f[:], bucket[:], 114.0, 8388608.0, ALU.add, ALU.mult)
            g_i = st_pool.tile([128, KT], I32, tag="g_i")
            nc.vector.tensor_copy(g_i[:], g_f[:])
            pw = g_i[:].bitcast(FP)
            # ---- scales: s = max(tmax*pw/7, 1e-9); r = 1/s ----
            ch_thr = st_pool.tile([128, KT], FP, tag="ch_thr")
            nc.vector.tensor_scalar(
                ch_thr[:], pw, tmax_b[:], None, ALU.mult)
            s_t = st_pool.tile([128, KT], FP, tag="s_t")
            nc.vector.tensor_scalar(
                s_t[:], ch_thr[:], C7, 1e-9, ALU.mult, ALU.max)
            r_t = st_pool.tile([128, KT], FP, tag="r_t")
            nc.vector.reciprocal(r_t[:], s_t[:])
            # ---- quantize: y = round(x*r) * pw   (fp8 e5m2, exact) ----
            # |x*r| <= 7 by construction so no clamping is needed
            for kt in range(KT):
                nc.scalar.activation(
                    xT[:, kt, :], xT[:, kt, :], ACT.Copy,
                    bias=C_MAGIC, scale=r_t[:, kt:kt + 1])
            for kt in range(KT):
                nc.vector.tensor_scalar(
                    y_c[:, kt, :], xT[:, kt, :], C_MAGIC,
                    g_i[:, kt:kt + 1].bitcast(FP), ALU.subtract, ALU.mult)

        def emit_m(c):
            y_c = y_tiles.pop(c)
            for th in range(2):
                tt_idx = c * 2 + th
                for vh in range(2):
                    pss = []
                    for q in range(4):
                        ps_mm = mps_pool.tile([128, 512], FP, tag="mps")
                        pss.append(ps_mm)
                    for kp in range(KP):
                        lhsT = y_c[:, 2 * kp:2 * kp + 2,
                                   th * 128:(th + 1) * 128]
                        for q in range(4):
                            vb = vh * 4 + q
                            nc.tensor.matmul(
                                pss[q][:, 0:VBS], lhsT,
                                qw_sb[:, 2 * kp:2 * kp + 2,
                                      vb * VBS:(vb + 1) * VBS],
                                start=(kp == 0), stop=(kp == KP - 1),
                                perf_mode=DR)
                    for q in range(4):
                        vb = vh * 4 + q
                        stg = stg_pool.tile([128, VBS], FP, tag="stg")
                        nc.vector.scalar_tensor_tensor(
                            stg[:], pss[q][:, 0:VBS], m7_all[:, c:c + 1],
                            sw_rep[:, vb * VBS:(vb + 1) * VBS],
                            op0=ALU.mult, op1=ALU.mult)
                        out_eng = nc.sync if q % 2 == 0 else nc.scalar
                        out_eng.dma_start(
                            out_d[tt_idx * 128:(tt_idx + 1) * 128,
                                  vb * VBS:(vb + 1) * VBS],
                            stg[:])

        emit_x(0)
        emit_x(1)
        for c in range(NCHUNK):
            emit_m(c)
            if c + 2 < NCHUNK:
                emit_x(c + 2)


_CACHED = None


def _build():
    global _CACHED
    if _CACHED is not None:
        return _CACHED
    nc = bacc.Bacc(
        "TRN2", target_bir_lowering=False, debug=False,
        enable_asserts=False, num_devices=NCORE)
    x_d = nc.dram_tensor("x", (T, H), FP, kind="ExternalInput").ap()
    w_d = nc.dram_tensor("w", (VSH, H), FP, kind="ExternalInput").ap()
    out_d = nc.dram_tensor("out", (T, VSH), FP, kind="ExternalOutput").ap()
    with tile.TileContext(nc) as tc:
        with ExitStack() as ctx:
            _emit(ctx, tc, x_d, w_d, out_d)
    nc.compile()
    _CACHED = nc
    return nc


def kernel(hidden_states: np.ndarray, lm_weight: np.ndarray) -> np.ndarray:
    b, t, h = hidden_states.shape
    assert (b * t, h) == (T, H) and lm_weight.shape == (V, H)
    x_full = np.ascontiguousarray(
        hidden_states.reshape(T, H).astype(np.float32))
    in_maps = []
    for c in range(NCORE):
        shard = np.ascontiguousarray(
            lm_weight[c * VSH:(c + 1) * VSH].astype(np.float32))
        in_maps.append({"x": x_full, "w": shard})
    nc = _build()
    res = run_bass_kernel_spmd(nc, in_maps, core_ids=list(range(NCORE)))
    outs = [res.results[c]["out"] for c in range(NCORE)]
    full = np.concatenate(outs, axis=1)
    return full.reshape(b, t, V)
